# revision 2
# baseline (speedup 1.0000x reference)
"""Trainium2 Bass kernel for nn_CRAMForCausalLM (v5).

8-way data-parallel over tokens (256 main + 16 halo per core; 0.5^17 < 8e-6
truncation).  bf16 GEMMs, f32r residual stream.  v2 structure (all epilogue
elementwise work on the Vector engine, LN mean-corrections as K=1 matmuls in
the GEMM accumulation groups) plus:
- rsqrt for LN on the Vector engine (quake + 2 Newton steps) and the psum
  broadcast consumed via scalar.copy, so the scalar engine never swaps
  activation tables at LN points (1.3us per swap, on the critical path).
- LN-stat matmuls interleave into producer loops; sigmoid is issued in the
  matmul loop while the scan chain lags one m-tile (no sigmoid bubbles).
- The redundant final LN (LN of an LN with identity scale) is skipped.
- Split pre-LM-head AllGather, LM weights prefetched under it, rr-major rhs
  loads so the first LM matmul group starts as soon as its columns land.
- LM head runs kc-major over 4 concurrent PSUM groups: one weight load
  serves 4 matmuls (weight-stationary reuse).
- Vocab-sharded LM head (4000 rows/core, padded 4096), FD=512.
"""

import numpy as np

import concourse.bass as bass
import concourse.bacc as bacc
import concourse.tile as tile
import concourse.mybir as mybir
import concourse.bass_utils as bass_utils
import os as _os

LAST_EXEC_NS = None


def _maybe_install_trace_hook():
    import contextlib, ctypes, sys, types
    if "antenv.axon_hooks" in sys.modules:
        return
    lib = ctypes.CDLL("/opt/axon/libaxon_pjrt.so")
    if not hasattr(lib, "axon_start_nrt_profile"):
        return
    lib.axon_start_nrt_profile.argtypes = [ctypes.POINTER(ctypes.c_int64), ctypes.c_size_t]
    lib.axon_start_nrt_profile.restype = ctypes.c_int64
    lib.axon_stop_nrt_profile.argtypes = [ctypes.c_char_p]
    lib.axon_stop_nrt_profile.restype = ctypes.c_int64

    @contextlib.contextmanager
    def _hook(output_dir, device_ids):
        import jax
        jax.devices()
        if device_ids:
            ids = (ctypes.c_int64 * len(device_ids))(*device_ids)
            rc = lib.axon_start_nrt_profile(ids, len(device_ids))
        else:
            rc = lib.axon_start_nrt_profile(None, 0)
        if rc != 0:
            raise RuntimeError(f"axon_start_nrt_profile rc={rc}")
        try:
            yield
        finally:
            lib.axon_stop_nrt_profile(str(output_dir).encode())

    mod = types.ModuleType("antenv.axon_hooks")
    mod.get_axon_ntff_profile_hook = lambda: _hook
    mod.set_axon_ntff_profile_hook = lambda h: None
    sys.modules["antenv.axon_hooks"] = mod

AF = mybir.ActivationFunctionType
OP = mybir.AluOpType

B, S, H, F, L, V = 2, 1024, 1024, 4096, 8, 32000
EPS = 1e-5
NCORES = 8
HALO = 16
TM = 256            # main tokens per core
T = TM + HALO       # 272 tokens processed per core
TPAD = 384          # padded to 3 x 128 for the embedding gather
KH = H // 128       # 8 k-chunks over H
MH = H // 128       # 8 m-tiles over H
MF = F // 128       # 32 m-tiles over F
VS = V // NCORES    # 4000 vocab rows per core
VSP = 4096          # padded vocab rows per core
TALL = B * S        # 2048 total tokens

f32 = mybir.dt.float32
f32r = mybir.dt.float32r
bf16 = mybir.dt.bfloat16
i32 = mybir.dt.int32

_compiled = {}


def _swz(w, kp=128, mf=128):
    """[K, M] -> [mt, kp, kc*mf] so lhsT tile (mt, kc) = sbuf[:, kc*mf:(kc+1)*mf]."""
    K, M = w.shape
    kc, mt = K // kp, M // mf
    return np.ascontiguousarray(
        w.reshape(kc, kp, mt, mf).transpose(2, 1, 0, 3).reshape(mt, kp, kc * mf)
    )


def _cols(v, mt, width=128):
    """[M] -> [width, mt] so column j is v[j*width:(j+1)*width]."""
    return np.ascontiguousarray(v.reshape(mt, width).T)


def _build(ln_scaled):
    nc = bacc.Bacc("TRN2", target_bir_lowering=False, debug=False,
                   num_devices=NCORES)
    wdt = bf16

    # ---- DRAM I/O ----
    ids_d = nc.dram_tensor("ids", [3, 128], i32, kind="ExternalInput")
    pos_d = nc.dram_tensor("pos", [3, 128, H], f32, kind="ExternalInput")
    wemb_d = nc.dram_tensor("wemb", [V, H], f32, kind="ExternalInput")
    retw_d = nc.dram_tensor("retw", [L, MH, 128, KH * 128], wdt, kind="ExternalInput")
    retb_d = nc.dram_tensor("retb", [L, 128, MH], f32, kind="ExternalInput")
    w1_d = nc.dram_tensor("w1", [L, MF, 128, KH * 128], wdt, kind="ExternalInput")
    b1_d = nc.dram_tensor("b1", [L, 128, MF], f32, kind="ExternalInput")
    w2_d = nc.dram_tensor("w2", [L, MH, 128, MF * 128], wdt, kind="ExternalInput")
    b2_d = nc.dram_tensor("b2", [L, 128, MH], f32, kind="ExternalInput")
    lmw_d = nc.dram_tensor("lmw", [VSP // 128, 128, KH * 128], wdt, kind="ExternalInput")
    mask_d = nc.dram_tensor("mask", [128, 1], f32, kind="ExternalInput")
    csret_d = nc.dram_tensor("csret", [L, 1, H], wdt, kind="ExternalInput")
    csw1_d = nc.dram_tensor("csw1", [L, 128, MF], f32, kind="ExternalInput")
    if ln_scaled:
        lns_d = nc.dram_tensor("lns", [2 * L + 2, 2, 128, MH], f32, kind="ExternalInput")
    out_d = nc.dram_tensor("logits", [VSP, TALL], f32, kind="ExternalOutput")

    with tile.TileContext(nc) as tc:
        with tc.tile_pool(name="per", bufs=1) as per, \
             tc.tile_pool(name="gpool", bufs=1) as gpool, \
             tc.tile_pool(name="lnout", bufs=2) as lnout:
            # persistent tiles
            xt = [per.tile([128, T], f32r, tag=f"xt{k}", name=f"xt{k}") for k in range(KH)]
            y1 = [per.tile([128, T], f32r, tag=f"y1{k}", name=f"y1{k}") for k in range(KH)]
            hres = [per.tile([128, T], f32r, tag=f"h{k}", name=f"h{k}") for k in range(KH)]
            fins = [per.tile([128, T], f32, tag=f"fin{k}", name=f"fin{k}") for k in range(KH)]
            sig = [per.tile([128, T], f32, tag=f"sg{k}", name=f"sg{k}") for k in range(KH)]
            yb1 = [per.tile([128, T], wdt, tag=f"yb1{k}", name=f"yb1{k}") for k in range(KH)]
            yb2 = [per.tile([128, T], wdt, tag=f"yb2{k}", name=f"yb2{k}") for k in range(KH)]
            sq1 = [per.tile([128, T], wdt, tag=f"sq1{k}", name=f"sq1{k}") for k in range(KH)]
            sq2 = [per.tile([128, T], wdt, tag=f"sq2{k}", name=f"sq2{k}") for k in range(KH)]
            g = [gpool.tile([128, T], wdt, tag=f"g{k}", name=f"g{k}") for k in range(MF)]
            ones_f = per.tile([128, 1], f32)
            nc.gpsimd.memset(ones_f[:], 1.0)
            ones = per.tile([128, 1], wdt)
            nc.vector.tensor_copy(ones[:], ones_f[:])
            onesr_f = per.tile([1, 128], f32)
            nc.gpsimd.memset(onesr_f[:], 1.0)
            onesr = per.tile([1, 128], f32r)
            nc.vector.tensor_copy(onesr[:], onesr_f[:])
            half = per.tile([128, T], f32)
            nc.gpsimd.memset(half[:], 0.5)
            # hneg: -1e30 on halo columns for sequence-start cores (mask=0),
            # 0 for mid-sequence cores; added to the sigmoid input so the
            # halo EMA contribution becomes sigmoid(-inf) = 0.
            mask = per.tile([128, 1], f32)
            nc.sync.dma_start(mask[:], mask_d.ap())
            hneg = per.tile([128, HALO], f32)
            nc.gpsimd.memset(hneg[:], 1.0)
            nc.vector.tensor_scalar(hneg[:], hneg[:], mask[:, :1], -1e30,
                                    OP.subtract, OP.mult)
            epsc = per.tile([128, 1], f32)
            nc.gpsimd.memset(epsc[:], EPS)
            epsc1 = per.tile([1, 1], f32)
            nc.gpsimd.memset(epsc1[:], EPS)
            dum = per.tile([1, 1], f32)
            nc.gpsimd.memset(dum[:], 1.0)
            ident = per.tile([128, 128], f32)
            from concourse.masks import make_identity
            make_identity(nc, ident[:])
            if ln_scaled:
                lnt = per.tile([128, (2 * L + 2) * 2 * MH], f32)
                nc.sync.dma_start(
                    lnt[:],
                    lns_d.ap().rearrange("a b p m -> p (a b m)"))
            else:
                lnt = None

            def ln_cols(slot):
                if lnt is None:
                    return None, None
                off = slot * 2 * MH
                return lnt[:, off:off + MH], lnt[:, off + MH:off + 2 * MH]

            def cast_sq(k, src, ybf, sq):
                nc.vector.tensor_copy(ybf[k][:], src[k][:].bitcast(f32))
                nc.vector.tensor_tensor(sq[k][:], ybf[k][:], ybf[k][:], OP.mult)

            # ---------- split LN stats ----------
            def stats_begin(ps_stat):
                p_st = ps_stat.tile([33, T], f32, tag="pst")
                return {"p_st": p_st, "p_sy": p_st[0:1, :], "p_sq": p_st[32:33, :]}

            def stats_accum(st, k, ybf, sq):
                nc.tensor.matmul(st["p_sy"], ones[:], ybf[k][:],
                                 start=(k == 0), stop=(k == KH - 1))
                nc.tensor.matmul(st["p_sq"], ones[:], sq[k][:],
                                 start=(k == 0), stop=(k == KH - 1))

            def stats_finish_vec(st, tmp):
                nm = lnout.tile([1, T], f32r, tag="nm", name="nm")
                nc.vector.tensor_scalar_mul(nm[:], st["p_sy"], -1.0 / H)
                v1 = tmp.tile([1, T], f32, tag="v1")
                nc.vector.tensor_scalar_mul(v1[:], st["p_sq"], 1.0 / H)
                m2 = tmp.tile([1, T], f32, tag="m2")
                nc.vector.tensor_tensor(m2[:], nm[:].bitcast(f32),
                                        nm[:].bitcast(f32), OP.mult)
                var = lnout.tile([1, T], f32r, tag="var", name="var")
                nc.vector.tensor_tensor(var[:], v1[:], m2[:], OP.subtract)
                nm_g = lnout.tile([1, T], wdt, tag="nmg", name="nm_g")
                nc.vector.tensor_copy(nm_g[:], nm[:].bitcast(f32))
                st["nm"], st["var"], st["nm_g"] = nm, var, nm_g

            def stats_finish_bcast(st, ps_bc):
                p_vb = ps_bc.tile([128, T], f32, tag="bc", name="p_vb")
                nc.tensor.matmul(p_vb[:], onesr[:], st["var"][:],
                                 start=True, stop=True)
                p_nmb = ps_bc.tile([128, T], f32, tag="bc", name="p_nmb")
                nc.tensor.matmul(p_nmb[:], onesr[:], st["nm"][:],
                                 start=True, stop=True)
                r_b = lnout.tile([128, T], f32, tag="rb", name="r_b")
                nc.scalar.activation(r_b[:], p_vb[:], AF.Abs_reciprocal_sqrt,
                                     bias=epsc[:])
                nmb_sb = lnout.tile([128, T], f32, tag="nmsb", name="nmb_sb")
                nc.scalar.copy(nmb_sb[:], p_nmb[:])
                st["r_b"], st["nmb_sb"] = r_b, nmb_sb

            def warm_rsqrt_table():
                # dummy rsqrt on a 1x1 tile: pulls the 1.3us activation-table
                # swap off the critical path (scalar engine is idle here)
                nc.scalar.activation(dum[:], dum[:], AF.Abs_reciprocal_sqrt,
                                     bias=epsc1[:])

            def ln_apply_k(tmp, yin, st, yout, slot, k):
                scol, bcol = ln_cols(slot)
                z = tmp.tile([128, T], f32, tag="z", name="z")
                nc.vector.tensor_tensor(z[:], yin[k][:].bitcast(f32),
                                        st["nmb_sb"][:], OP.add)
                if scol is None:
                    nc.vector.tensor_tensor(yout[k][:], z[:],
                                            st["r_b"][:], OP.mult)
                else:
                    z2 = tmp.tile([128, T], f32, tag="z2", name="z2")
                    nc.vector.tensor_tensor(z2[:], z[:], st["r_b"][:], OP.mult)
                    nc.vector.tensor_scalar(
                        yout[k][:], z2[:],
                        scol[:, k:k + 1], bcol[:, k:k + 1], OP.mult, OP.add)

            # ================= Embedding =================
            with tc.tile_pool(name="emb", bufs=1) as ep, \
                 tc.tile_pool(name="pse", bufs=3, space="PSUM") as pse, \
                 tc.tile_pool(name="dramw", bufs=1, space="DRAM") as dramw:
                # tiny warm-up AllGather to absorb collective setup cost
                win = dramw.tile([128, 4], f32)
                nc.sync.dma_start(win[:], ident[:, :4])
                wout = dramw.tile([NCORES, 128, 4], f32, addr_space="Shared")
                nc.gpsimd.collective_compute(
                    "AllGather", OP.bypass,
                    replica_groups=[list(range(NCORES))],
                    ins=[win.opt()], outs=[wout.opt()])
                idxs, gts, pts = [], [], []
                for c in range(3):
                    idx = ep.tile([128, 1], i32, tag=f"idx{c}", name=f"idx{c}")
                    nc.sync.dma_start(idx[:], ids_d.ap()[c][:, None])
                    idxs.append(idx)
                for c in range(3):
                    gt = ep.tile([128, H], f32, tag=f"gt{c}", name=f"gt{c}")
                    nc.gpsimd.indirect_dma_start(
                        out=gt[:], out_offset=None, in_=wemb_d.ap(),
                        in_offset=bass.IndirectOffsetOnAxis(ap=idxs[c][:, :1], axis=0))
                    gts.append(gt)
                    pt = ep.tile([128, H], f32, tag=f"pt{c}", name=f"pt{c}")
                    nc.sync.dma_start(pt[:], pos_d.ap()[c])
                    pts.append(pt)
                for c in range(3):
                    nc.vector.tensor_tensor(gts[c][:], gts[c][:], pts[c][:],
                                            OP.add)
                    cnt = T - 256 if c == 2 else 128
                    for k in range(KH):
                        ptr = pse.tile([128, 128], f32, tag="ptr")
                        nc.tensor.transpose(ptr[:], gts[c][:, k * 128:(k + 1) * 128],
                                            ident[:])
                        nc.vector.tensor_copy(
                            y1[k][:, c * 128:c * 128 + cnt], ptr[:, :cnt])

            # ================= Layers =================
            with tc.tile_pool(name="wret", bufs=6) as wret, \
                 tc.tile_pool(name="w1p", bufs=4) as w1p, \
                 tc.tile_pool(name="w2p", bufs=4) as w2p, \
                 tc.tile_pool(name="bias", bufs=2) as biasp, \
                 tc.tile_pool(name="tmp", bufs=3) as tmp, \
                 tc.tile_pool(name="psmm", bufs=4, space="PSUM") as psmm, \
                 tc.tile_pool(name="psst", bufs=2, space="PSUM") as ps_stat, \
                 tc.tile_pool(name="psbc", bufs=2, space="PSUM") as ps_bc:

                # embedding-LN stats (interleaved casts + accums)
                st2 = stats_begin(ps_stat)
                for k in range(KH):
                    cast_sq(k, y1, yb2, sq2)
                    stats_accum(st2, k, yb2, sq2)
                stats_finish_vec(st2, tmp)

                def load_biases(l):
                    bs = {}
                    bs["retb"] = biasp.tile([128, MH], f32, tag="retb", name="retb")
                    nc.sync.dma_start(bs["retb"][:], retb_d.ap()[l])
                    bs["b1"] = biasp.tile([128, MF], f32, tag="b1", name="b1")
                    nc.sync.dma_start(bs["b1"][:], b1_d.ap()[l])
                    bs["b2"] = biasp.tile([128, MH], f32, tag="b2", name="b2")
                    nc.sync.dma_start(bs["b2"][:], b2_d.ap()[l])
                    bs["csr"] = biasp.tile([1, H], wdt, tag="csr", name="csr")
                    nc.sync.dma_start(bs["csr"][:], csret_d.ap()[l])
                    bs["cs1"] = biasp.tile([128, MF], f32, tag="cs1", name="cs1")
                    nc.sync.dma_start(bs["cs1"][:], csw1_d.ap()[l])
                    return bs

                bs = load_biases(0)
                PFW = 3

                def fetch_wret(l, mt):
                    wt = wret.tile([128, KH * 128], wdt, tag="wret", name="wret")
                    nc.sync.dma_start(wt[:], retw_d.ap()[l, mt])
                    return wt

                def fetch_w1(l, mt):
                    wt = w1p.tile([128, KH * 128], wdt, tag="w1", name="w1")
                    nc.sync.dma_start(wt[:], w1_d.ap()[l, mt])
                    return wt

                def fetch_w2(l, mt):
                    wt = w2p.tile([128, MF * 128], wdt, tag="w2", name="w2")
                    nc.sync.dma_start(wt[:], w2_d.ap()[l, mt])
                    return wt

                pfr = fetch_wret(0, 0)
                for l in range(L):
                    retb, b1, b2, csr, cs1 = (bs["retb"], bs["b1"], bs["b2"],
                                              bs["csr"], bs["cs1"])

                    # --- retention phase ---
                    st1 = stats_begin(ps_stat)

                    def ret_tail(j):
                        # EMA scan on raw sigmoid out: out'[t]=s[t]+.5*out'[t-1]
                        # => xi = out'/2 ; y1 = xi + xt
                        stt = tmp.tile([128, T], f32, tag="scan", name="scan")
                        nc.vector.tensor_tensor_scan(
                            stt[:], half[:], sig[j][:], 0.0, OP.mult, OP.add)
                        nc.vector.scalar_tensor_tensor(
                            y1[j][:], stt[:], 0.5,
                            xt[j][:].bitcast(f32), OP.mult, OP.add)
                        cast_sq(j, y1, yb1, sq1)
                        stats_accum(st1, j, yb1, sq1)

                    for mt in range(MH):
                        wt = pfr if mt == 0 else fetch_wret(l, mt)
                        if mt == 0:
                            pf1 = fetch_w1(l, 0)
                        ps = psmm.tile([128, T], f32, tag="mm")
                        for kc in range(KH):
                            nc.tensor.matmul(
                                ps[:], wt[:, kc * 128:(kc + 1) * 128], yb2[kc][:],
                                start=(kc == 0), stop=False)
                        if mt == 0:
                            stats_finish_bcast(st2, ps_bc)
                        nc.tensor.matmul(
                            ps[:], csr[:, mt * 128:(mt + 1) * 128],
                            st2["nm_g"][:], start=False, stop=True)
                        ln_apply_k(tmp, y1, st2, xt, 2 * l, mt)
                        nc.vector.tensor_tensor(fins[mt][:], ps[:],
                                                st2["r_b"][:], OP.mult)
                        nc.vector.tensor_tensor(fins[mt][:, :HALO],
                                                fins[mt][:, :HALO],
                                                hneg[:], OP.add)
                        nc.scalar.activation(sig[mt][:], fins[mt][:],
                                             AF.Sigmoid,
                                             bias=retb[:, mt:mt + 1])
                        if mt == MH - 1:
                            warm_rsqrt_table()
                        if mt >= 1:
                            ret_tail(mt - 1)
                    ret_tail(MH - 1)
                    stats_finish_vec(st1, tmp)

                    # --- FFN1 phase (fused with LN1 via corr matmul) ---
                    for mt in range(MF):
                        wt = pf1 if mt == 0 else fetch_w1(l, mt)
                        if mt == 0:
                            pf2 = fetch_w2(l, 0)
                        ps = psmm.tile([128, T], f32, tag="mm")
                        for kc in range(KH):
                            nc.tensor.matmul(
                                ps[:], wt[:, kc * 128:(kc + 1) * 128], yb1[kc][:],
                                start=(kc == 0), stop=(kc == KH - 1))
                        if mt == 0:
                            stats_finish_bcast(st1, ps_bc)
                        if mt < KH:
                            ln_apply_k(tmp, y1, st1, hres, 2 * l + 1, mt)
                        # LN mean-correction fused on vector: (nmb*cs + ps)*r
                        fin = tmp.tile([128, T], f32, tag="epf", name="epf")
                        nc.vector.scalar_tensor_tensor(
                            fin[:], st1["nmb_sb"][:], cs1[:, mt:mt + 1],
                            ps[:], OP.mult, OP.add)
                        nc.vector.tensor_tensor(fin[:], fin[:], st1["r_b"][:],
                                                OP.mult)
                        nc.scalar.activation(g[mt][:], fin[:],
                                             AF.Gelu_apprx_tanh,
                                             bias=b1[:, mt:mt + 1])
                        if mt == MF - 1 and l + 1 < L:
                            bs = load_biases(l + 1)

                    # --- FFN2 phase ---
                    st2 = stats_begin(ps_stat)
                    for mt in range(MH):
                        wt = pf2 if mt == 0 else fetch_w2(l, mt)
                        if mt == 0 and l + 1 < L:
                            pfr = fetch_wret(l + 1, 0)
                        ps = psmm.tile([128, T], f32, tag="mm")
                        for kc in range(MF):
                            nc.tensor.matmul(
                                ps[:], wt[:, kc * 128:(kc + 1) * 128], g[kc][:],
                                start=(kc == 0), stop=(kc == MF - 1))
                        # y1 = (ffn + b2) + h    (becomes LN2 input)
                        nc.vector.scalar_tensor_tensor(
                            y1[mt][:], ps[:], b2[:, mt:mt + 1],
                            hres[mt][:].bitcast(f32), OP.add, OP.add)
                        cast_sq(mt, y1, yb2, sq2)
                        stats_accum(st2, mt, yb2, sq2)
                        if mt == 2:
                            warm_rsqrt_table()
                    stats_finish_vec(st2, tmp)

                # ---- tail: LN2(l=7) -> xf directly (the final LN of an LN
                # with identity scale/bias is an identity to ~eps/2) ----
                xf = yb1  # reuse bf16 tiles
                stats_finish_bcast(st2, ps_bc)
                if not ln_scaled:
                    for k in range(KH):
                        z = tmp.tile([128, T], f32, tag="z", name="z")
                        nc.vector.tensor_tensor(z[:], y1[k][:].bitcast(f32),
                                                st2["nmb_sb"][:], OP.add)
                        nc.vector.tensor_tensor(xf[k][:], z[:],
                                                st2["r_b"][:], OP.mult)
                else:
                    for k in range(KH):
                        ln_apply_k(tmp, y1, st2, xt, 2 * L, k)
                    stf = stats_begin(ps_stat)
                    for k in range(KH):
                        cast_sq(k, xt, yb2, sq2)
                        stats_accum(stf, k, yb2, sq2)
                    stats_finish_vec(stf, tmp)
                    stats_finish_bcast(stf, ps_bc)
                    for k in range(KH):
                        ln_apply_k(tmp, xt, stf, xf, 2 * L + 1, k)

            # ====== AllGather (split) + vocab-sharded LM head with filler ======
            with tc.tile_pool(name="dram", bufs=1, space="DRAM") as dramp, \
                 tc.tile_pool(name="lmx", bufs=1) as lmx, \
                 tc.tile_pool(name="lmw", bufs=6) as lmwp, \
                 tc.tile_pool(name="lmo", bufs=6) as lmo, \
                 tc.tile_pool(name="pslm", bufs=2, space="PSUM") as pslm:
                KHH = KH // 2
                bnc0 = dramp.tile([KHH * 128, TM], wdt)
                bnc1 = dramp.tile([KHH * 128, TM], wdt)
                xg0 = dramp.tile([NCORES, KHH * 128, TM], wdt, addr_space="Shared")
                xg1 = dramp.tile([NCORES, KHH * 128, TM], wdt, addr_space="Shared")
                for k in range(KHH):
                    nc.sync.dma_start(bnc0[k * 128:(k + 1) * 128, :],
                                      xf[k][:, HALO:T])
                nc.gpsimd.collective_compute(
                    "AllGather", OP.bypass,
                    replica_groups=[list(range(NCORES))],
                    ins=[bnc0.opt()], outs=[xg0.opt()])
                for k in range(KHH):
                    nc.sync.dma_start(bnc1[k * 128:(k + 1) * 128, :],
                                      xf[KHH + k][:, HALO:T])
                nc.gpsimd.collective_compute(
                    "AllGather", OP.bypass,
                    replica_groups=[list(range(NCORES))],
                    ins=[bnc1.opt()], outs=[xg1.opt()])

                NRR = TALL // 512        # 4 psum column groups
                rhs = [[None] * NRR for _ in range(KH)]
                for rr in range(NRR):
                    for kc in range(KH):
                        xg = xg0 if kc < KHH else xg1
                        ko = kc if kc < KHH else kc - KHH
                        t_ = lmx.tile([128, 512], wdt, tag=f"rhs{kc}_{rr}",
                                      name=f"rhs{kc}_{rr}")
                        nc.sync.dma_start(
                            t_[:, 0:TM],
                            xg[2 * rr, ko * 128:(ko + 1) * 128, :])
                        nc.sync.dma_start(
                            t_[:, TM:512],
                            xg[2 * rr + 1, ko * 128:(ko + 1) * 128, :])
                        rhs[kc][rr] = t_

                NPF = 6
                wts = []
                for mt in range(NPF):
                    wt = lmwp.tile([128, KH * 128], wdt, tag="lmw", name="lmw")
                    nc.sync.dma_start(wt[:], lmw_d.ap()[mt])
                    wts.append(wt)
                for mt in range(VSP // 128):
                    if mt < NPF:
                        wt = wts[mt]
                    else:
                        wt = lmwp.tile([128, KH * 128], wdt, tag="lmw", name="lmw")
                        nc.sync.dma_start(wt[:], lmw_d.ap()[mt])
                    pss = [pslm.tile([128, 512], f32, tag=f"lm{rr}",
                                     name=f"lm{rr}") for rr in range(NRR)]
                    for kc in range(KH):
                        for rr in range(NRR):
                            nc.tensor.matmul(
                                pss[rr][:], wt[:, kc * 128:(kc + 1) * 128],
                                rhs[kc][rr][:],
                                start=(kc == 0), stop=(kc == KH - 1))
                    for rr in range(NRR):
                        ob = lmo.tile([128, 512], f32, tag="ob")
                        nc.scalar.copy(ob[:], pss[rr][:])
                        nc.sync.dma_start(
                            out_d.ap()[mt * 128:(mt + 1) * 128,
                                       rr * 512:(rr + 1) * 512],
                            ob[:])

    nc.compile()
    return nc


def _prep_inputs(inputs, ln_scaled):
    import ml_dtypes
    wdtype = ml_dtypes.bfloat16
    ids = np.asarray(inputs["input_ids"], np.int32)          # [B, S]
    retw_raw = [np.asarray(inputs["ret_W"][l], np.float32) for l in range(L)]
    w1_raw = [np.asarray(inputs["ffn_W1"][l], np.float32) for l in range(L)]
    retb_raw = [np.asarray(inputs["ret_b"][l], np.float32) for l in range(L)]
    b1_raw = [np.asarray(inputs["ffn_b1"][l], np.float32) for l in range(L)]
    if ln_scaled:
        # fold LN scale/bias of the LN feeding each fused GEMM into W / bias
        for l in range(L):
            s_in = (np.asarray(inputs["emb_ln_s"], np.float32) if l == 0
                    else np.asarray(inputs["ln2_s"][l - 1], np.float32))
            b_in = (np.asarray(inputs["emb_ln_b"], np.float32) if l == 0
                    else np.asarray(inputs["ln2_b"][l - 1], np.float32))
            retb_raw[l] = retb_raw[l] + b_in @ retw_raw[l]
            retw_raw[l] = retw_raw[l] * s_in[:, None]
            s1 = np.asarray(inputs["ln1_s"][l], np.float32)
            b1_ = np.asarray(inputs["ln1_b"][l], np.float32)
            b1_raw[l] = b1_raw[l] + b1_ @ w1_raw[l]
            w1_raw[l] = w1_raw[l] * s1[:, None]
    csret = np.stack([w.sum(0) for w in retw_raw]).reshape(L, 1, H).astype(wdtype)
    csw1 = np.stack([_cols(w.sum(0), MF) for w in w1_raw])  # [L,128,MF] f32
    retw = np.stack([_swz(w) for w in retw_raw]).astype(wdtype)
    w1 = np.stack([_swz(w) for w in w1_raw]).astype(wdtype)
    w2 = np.stack([_swz(np.asarray(inputs["ffn_W2"][l], np.float32))
                   for l in range(L)]).astype(wdtype)
    retb = np.stack([_cols(v, MH) for v in retb_raw])
    b1 = np.stack([_cols(v, MF) for v in b1_raw])
    b2 = np.stack([_cols(np.asarray(inputs["ffn_b2"][l], np.float32), MH)
                   for l in range(L)])
    lmw_full = np.asarray(inputs["lm_W"], np.float32)         # [H, V]
    pos_emb = np.asarray(inputs["pos_emb"], np.float32)       # [S, H]
    wemb = np.ascontiguousarray(np.asarray(inputs["word_emb"], np.float32))

    common = {
        "wemb": wemb, "retw": retw, "retb": retb,
        "w1": w1, "b1": b1, "w2": w2, "b2": b2,
        "csret": csret, "csw1": csw1,
    }
    if ln_scaled:
        slots = [( np.asarray(inputs["emb_ln_s"], np.float32),
                   np.asarray(inputs["emb_ln_b"], np.float32))]
        for l in range(L):
            slots.append((np.asarray(inputs["ln1_s"][l], np.float32),
                          np.asarray(inputs["ln1_b"][l], np.float32)))
            slots.append((np.asarray(inputs["ln2_s"][l], np.float32),
                          np.asarray(inputs["ln2_b"][l], np.float32)))
        slots.append((np.asarray(inputs["fin_ln_s"], np.float32),
                      np.asarray(inputs["fin_ln_b"], np.float32)))
        lns = np.stack([np.stack([_cols(s, MH), _cols(b, MH)]) for s, b in slots])
        common["lns"] = lns

    in_maps = []
    for c in range(NCORES):
        b = c // (NCORES // B)
        s0 = TM * (c % (NCORES // B))
        if s0 == 0:
            hids = ids[b, 0:HALO]
            hpos = np.arange(HALO)
        else:
            hids = ids[b, s0 - HALO:s0]
            hpos = np.arange(s0 - HALO, s0)
        cids = np.concatenate([hids, ids[b, s0:s0 + TM],
                               np.zeros(TPAD - T, np.int32)]).astype(np.int32)
        cpos = np.concatenate([hpos, np.arange(s0, s0 + TM),
                               np.zeros(TPAD - T, np.int64)])
        pos = pos_emb[cpos].reshape(3, 128, H)
        lmw_c = np.zeros((H, VSP), np.float32)
        lmw_c[:, :VS] = lmw_full[:, c * VS:(c + 1) * VS]
        m = dict(common)
        m["mask"] = np.full((128, 1), 0.0 if s0 == 0 else 1.0, np.float32)
        m["ids"] = cids.reshape(3, 128)
        m["pos"] = np.ascontiguousarray(pos)
        m["lmw"] = _swz(lmw_c).astype(wdtype)
        in_maps.append(m)
    return in_maps


def kernel(**inputs):
    trivial = all(
        np.allclose(np.asarray(inputs[k]), 1.0) for k in
        ("emb_ln_s", "ln1_s", "ln2_s", "fin_ln_s")
    ) and all(
        np.allclose(np.asarray(inputs[k]), 0.0) for k in
        ("emb_ln_b", "ln1_b", "ln2_b", "fin_ln_b")
    )
    ln_scaled = not trivial

    if ln_scaled not in _compiled:
        _compiled[ln_scaled] = _build(ln_scaled)
    nc = _compiled[ln_scaled]

    in_maps = _prep_inputs(inputs, ln_scaled)
    trace = bool(_os.environ.get("KERNEL_TRACE"))
    if trace:
        _maybe_install_trace_hook()
    res = bass_utils.run_bass_kernel_spmd(
        nc, in_maps, core_ids=list(range(NCORES)), trace=trace)
    global LAST_EXEC_NS
    LAST_EXEC_NS = res.exec_time_ns

    logits = np.empty((TALL, V), np.float32)
    for c in range(NCORES):
        logits[:, c * VS:(c + 1) * VS] = res.results[c]["logits"][:VS, :].T
    return logits.reshape(B, S, V)
